# revision 2
# baseline (speedup 1.0000x reference)
"""Trainium2 Bass kernel v2 for nn_ConvDipModel.

Structure vs v1 (baseline):
- BN1 stats computed analytically on the HOST from x's second moment
  (R = X^T X is 64x64; stats of the linear conv output follow exactly).
  The BN1 shift is folded into a 65th (ones) row of the conv matrix M-hat,
  the BN1 scale into the fc1 weights (per-column), handling negative scales
  by sign-flipping M-hat columns. Conv's PSUM->SBUF copy is then a pure
  relu, one op on any engine.
- Matmul loops ordered kc-outer / bj-inner with accumulation groups
  interleaved across 4 PSUM banks so group-boundary drains hide behind
  other banks' streams; stationary weights repeat across bj (weight-FIFO
  friendly).
- Copies round-robin over scalar/vector/gpsimd engines.
- BN2 keeps the exact global AllReduce (per-shard stats fail the 2e-2
  gate); stats ride a [128, 8] layout so readback is one DMA.
- fc2 output-major in fp16 with per-partition bias folded into the copy.
"""

import sys

import ml_dtypes
import numpy as np

sys.path.insert(0, "/opt/trn_rl_repo")

import concourse.bacc as bacc
import concourse.mybir as mybir
import concourse.tile as tile
from concourse.bass_utils import run_bass_kernel_spmd

F32 = mybir.dt.float32
F16 = mybir.dt.float16
BF16 = mybir.dt.bfloat16
AF = mybir.ActivationFunctionType
ALU = mybir.AluOpType

N_CORES = 8
CORE_IDS = list(range(N_CORES))
B, C_IN, OUT = 16384, 64, 5124
GRID = 12
NPIX = GRID * GRID  # 144
NCH = 8
YF = NCH * NPIX     # 1152
H1 = 512
BL = B // N_CORES   # 2048
EPS = 1e-5
NOJ = (OUT + 127) // 128  # 41

_CACHE = {}
TRACE = False
TRACE_DIR = None


class _Rot:
    """Round-robin copy-engine dispatcher: scalar / vector (gpsimd cannot
    read PSUM)."""

    def __init__(self, nc):
        self.nc = nc
        self.i = 0

    def _next(self):
        e = self.i % 2
        self.i += 1
        return e

    def relu_copy(self, out, in_):
        e = self._next()
        nc = self.nc
        if e == 0:
            nc.scalar.activation(out, in_, AF.Relu)
        else:
            nc.vector.tensor_scalar_max(out, in_, 0.0)

    def copy(self, out, in_):
        e = self._next()
        nc = self.nc
        if e == 0:
            nc.scalar.copy(out, in_)
        else:
            nc.vector.tensor_copy(out, in_)

    def bias_copy(self, out, in_, bias):
        # out = in_ + bias  ([128,1] per-partition bias), with dtype cast
        e = self._next()
        nc = self.nc
        if e == 0:
            nc.scalar.activation(out, in_, AF.Identity, bias=bias)
        else:
            nc.vector.tensor_scalar_add(out, in_, bias)


def _build():
    nc = bacc.Bacc("TRN2", target_bir_lowering=False, debug=False, num_devices=N_CORES)

    xh_d = nc.dram_tensor("xhT", [C_IN + 1, BL], BF16, kind="ExternalInput").ap()
    mh_d = nc.dram_tensor("mh", [C_IN + 1, YF], BF16, kind="ExternalInput").ap()
    w1_d = nc.dram_tensor("fc1wT", [YF, H1], BF16, kind="ExternalInput").ap()
    w2_d = nc.dram_tensor("fc2wT", [H1, OUT], BF16, kind="ExternalInput").ap()
    b2t_d = nc.dram_tensor("fc2bt", [128, NOJ], F32, kind="ExternalInput").ap()
    g2_d = nc.dram_tensor("bn2g", [128, 4], F32, kind="ExternalInput").ap()
    be2_d = nc.dram_tensor("bn2b", [128, 4], F32, kind="ExternalInput").ap()
    o_d = nc.dram_tensor("out", [OUT, BL], F16, kind="ExternalOutput").ap()

    with tile.TileContext(nc) as tc:
        with (
            tc.tile_pool(name="const", bufs=1) as cp,
            tc.tile_pool(name="acts", bufs=1) as ap_,
            tc.tile_pool(name="work", bufs=4) as wp,
            tc.tile_pool(name="ps", bufs=1, space="PSUM") as ps,
            tc.tile_pool(name="dram", bufs=1, space="DRAM") as dp,
        ):
            rot = _Rot(nc)

            # -------- constants, in need-order --------
            mh_sb = cp.tile([C_IN + 1, YF], BF16, tag="mh")
            nc.sync.dma_start(out=mh_sb[:], in_=mh_d[:])
            xh_sb = cp.tile([C_IN + 1, BL], BF16, tag="xh")
            nc.sync.dma_start(out=xh_sb[:], in_=xh_d[:])
            w1_sb = []
            for kc in range(9):
                t = cp.tile([128, H1], BF16, tag=f"w1_{kc}", name=f"w1_{kc}")
                nc.sync.dma_start(out=t[:], in_=w1_d[kc * 128 : (kc + 1) * 128, :])
                w1_sb.append(t)
            g2_sb = cp.tile([128, 4], F32, tag="bn2g")
            nc.sync.dma_start(out=g2_sb[:], in_=g2_d[:])
            be2_sb = cp.tile([128, 4], F32, tag="bn2b")
            nc.sync.dma_start(out=be2_sb[:], in_=be2_d[:])
            b2t_sb = cp.tile([128, NOJ], F32, tag="b2t")
            nc.sync.dma_start(out=b2t_sb[:], in_=b2t_d[:])
            w2_sb = []
            for kc in range(4):
                t = cp.tile([128, OUT], BF16, tag=f"w2_{kc}", name=f"w2_{kc}")
                nc.sync.dma_start(out=t[:], in_=w2_d[kc * 128 : (kc + 1) * 128, :])
                w2_sb.append(t)

            # -------- persistent activations --------
            yT = [ap_.tile([128, BL], BF16, tag=f"yT{k}", name=f"yT{k}") for k in range(9)]
            hT = [ap_.tile([128, BL], BF16, tag=f"hT{n}", name=f"hT{n}") for n in range(4)]

            # -------- warmup AllReduce: pays the cc-stream cold cost early --------
            ar_in = dp.tile([128, 8], F32, tag="ar_in")
            ar_out = dp.tile([128, 8], F32, tag="ar_out", addr_space="Shared")
            aw_in = dp.tile([1, 8], F32, tag="aw_in")
            aw_out = dp.tile([1, 8], F32, tag="aw_out", addr_space="Shared")
            nc.gpsimd.collective_compute(
                "AllReduce", ALU.add, replica_groups=[CORE_IDS],
                ins=[aw_in.opt()], outs=[aw_out.opt()],
            )

            # -------- PE pre-warm: dummy matmuls on memset data during DMA wait --------
            warm = cp.tile([128, 640], BF16, tag="warm")
            nc.vector.memset(warm[:], 0.0)
            for i in range(14):
                wps = ps.tile([128, 512], F32, tag="mmA", bufs=4, name=f"warm{i}")
                nc.tensor.matmul(
                    wps[:], warm[:, 0:128], warm[:, 128:640], start=True, stop=True
                )

            # -------- conv + fc1 pipelined over batch columns --------
            # conv block bj: 9 matmuls + relu copies -> yT[:, bj]
            # fc1 block bj: 36 matmuls (kc-outer nj-inner) + stats/copies -> hT[:, bj]
            # order: c0 c1 f0 c2 f1 c3 f2 f3 keeps copies off the critical path.
            bnst = [
                wp.tile([128, 4, 6], F32, tag=f"bnst{nj}", name=f"bnst{nj}")
                for nj in range(4)
            ]
            stall = wp.tile([128, 8], F32, tag="stall")

            def conv_block(bj):
                banks = [
                    ps.tile([128, 512], F32, tag="mmA", bufs=4, name=f"c{kc}_{bj}")
                    for kc in range(9)
                ]
                for kc in range(9):
                    nc.tensor.matmul(
                        banks[kc][:], mh_sb[:, kc * 128 : (kc + 1) * 128],
                        xh_sb[:, bj * 512 : (bj + 1) * 512],
                        start=True, stop=True,
                    )
                    rot.relu_copy(yT[kc][:, bj * 512 : (bj + 1) * 512], banks[kc][:])

            def fc1_block(bj):
                banks = [
                    ps.tile([128, 512], F32, tag="mmB", bufs=4, name=f"f{nj}_{bj}")
                    for nj in range(4)
                ]
                for kc in range(9):
                    for nj in range(4):
                        nc.tensor.matmul(
                            banks[nj][:], w1_sb[kc][:, nj * 128 : (nj + 1) * 128],
                            yT[kc][:, bj * 512 : (bj + 1) * 512],
                            start=(kc == 0), stop=(kc == 8),
                        )
                for nj in range(4):
                    nc.vector.bn_stats(out=bnst[nj][:, bj, :], in_=banks[nj][:])
                    rot.copy(hT[nj][:, bj * 512 : (bj + 1) * 512], banks[nj][:])
                if bj == 3:
                    for nj in range(4):
                        mv = wp.tile([128, 2], F32, tag="mv", name=f"mv{nj}")
                        nc.vector.bn_aggr(out=mv[:], in_=bnst[nj][:])
                        # means in stall cols 0-3, E[h^2] in cols 4-7
                        nc.vector.tensor_copy(stall[:, nj : nj + 1], mv[:, 0:1])
                        nc.vector.tensor_mul(
                            stall[:, 4 + nj : 5 + nj], mv[:, 0:1], mv[:, 0:1]
                        )
                        nc.vector.tensor_add(
                            stall[:, 4 + nj : 5 + nj], stall[:, 4 + nj : 5 + nj],
                            mv[:, 1:2],
                        )
                    nc.sync.dma_start(out=ar_in[:], in_=stall[:])

            conv_block(0)
            conv_block(1)
            fc1_block(0)
            conv_block(2)
            fc1_block(1)
            conv_block(3)
            fc1_block(2)
            fc1_block(3)

            # -------- AllReduce of BN2 sums (4 KB) --------
            nc.gpsimd.collective_compute(
                "AllReduce", ALU.add, replica_groups=[CORE_IDS],
                ins=[ar_in.opt()], outs=[ar_out.opt()],
            )
            gs = wp.tile([128, 8], F32, tag="gs")
            nc.sync.dma_start(out=gs[:], in_=ar_out[:])
            inv_n = 1.0 / N_CORES
            tw = wp.tile([128, 4, 4], F32, tag="tw")
            sc = wp.tile([128, 2, 4], F32, tag="sc")
            # all 4 nj chunks at once (means cols 0-3, E[h^2] cols 4-7)
            nc.vector.tensor_scalar_mul(tw[:, 0, :], gs[:, 0:4], inv_n)     # mean
            nc.vector.tensor_scalar_mul(tw[:, 1, :], gs[:, 4:8], inv_n)     # E[h^2]
            nc.vector.tensor_mul(tw[:, 2, :], tw[:, 0, :], tw[:, 0, :])     # mean^2
            nc.vector.tensor_sub(tw[:, 3, :], tw[:, 1, :], tw[:, 2, :])     # var
            nc.vector.tensor_scalar_add(tw[:, 3, :], tw[:, 3, :], EPS)
            nc.scalar.sqrt(tw[:, 3, :], tw[:, 3, :])
            nc.vector.reciprocal(tw[:, 3, :], tw[:, 3, :])                  # rstd
            nc.vector.tensor_mul(sc[:, 0, :], g2_sb[:], tw[:, 3, :])        # scale
            nc.vector.tensor_mul(tw[:, 2, :], tw[:, 0, :], sc[:, 0, :])     # mean*scale
            nc.vector.tensor_sub(sc[:, 1, :], be2_sb[:], tw[:, 2, :])       # shift
            # norm + relu in place on hT. gpsimd must NOT touch this (a
            # tensor-op after collective_compute forces a ~27us ucode library
            # reload). Each chunk is split: scalar (1-op activation) takes the
            # left half, vector (2-op) the right, in fc2's kc consumption
            # order, so fc2's matmul stream can start after the first chunk.
            for nj in range(4):
                nc.scalar.activation(
                    hT[nj][:, 0:1024], hT[nj][:, 0:1024], AF.Relu,
                    bias=sc[:, 1, nj : nj + 1], scale=sc[:, 0, nj : nj + 1],
                )
                nc.vector.tensor_scalar(
                    hT[nj][:, 1024:BL], hT[nj][:, 1024:BL],
                    sc[:, 0, nj : nj + 1], sc[:, 1, nj : nj + 1],
                    ALU.mult, ALU.add,
                )
                nc.vector.tensor_scalar_max(
                    hT[nj][:, 1024:BL], hT[nj][:, 1024:BL], 0.0
                )

            # -------- fc2 output-major + bias, fp16 out --------
            for oj in range(NOJ):
                mo = min(128, OUT - oj * 128)
                tg = "mmA" if oj % 2 == 0 else "mmB"
                banks = [
                    ps.tile([128, 512], F32, tag=tg, bufs=4, name=f"o{oj}_{bj}")
                    for bj in range(4)
                ]
                for kc in range(4):
                    for bj in range(4):
                        nc.tensor.matmul(
                            banks[bj][:mo, :], w2_sb[kc][:, oj * 128 : oj * 128 + mo],
                            hT[kc][:, bj * 512 : (bj + 1) * 512],
                            start=(kc == 0), stop=(kc == 3),
                        )
                osb = wp.tile([128, BL], F16, tag="osb", bufs=3, name=f"os{oj}")
                for bj in range(4):
                    rot.bias_copy(
                        osb[:mo, bj * 512 : (bj + 1) * 512], banks[bj][:mo, :],
                        b2t_sb[:mo, oj : oj + 1],
                    )
                nc.sync.dma_start(out=o_d[oj * 128 : oj * 128 + mo, :], in_=osb[:mo, :])
    nc.compile()
    return nc


def _host_prep(x, interp_W, head_mask, conv_w, bn1_g, bn1_b, fc1_w, fc2_w, fc2_b):
    bf = ml_dtypes.bfloat16
    # fold interp + 3x3 conv into M [64, 1152]
    W2 = np.zeros((NPIX, YF), dtype=np.float64)
    cw = conv_w.astype(np.float64)
    for o in range(NCH):
        for py in range(GRID):
            for px in range(GRID):
                pcol = o * NPIX + py * GRID + px
                for dy in range(3):
                    for dx in range(3):
                        qy, qx = py + dy - 1, px + dx - 1
                        if 0 <= qy < GRID and 0 <= qx < GRID:
                            W2[qy * GRID + qx, pcol] += cw[o, 0, dy, dx]
    M = (interp_W.astype(np.float64) * head_mask.astype(np.float64)[:, None]).T @ W2
    Mq = M.astype(np.float32).astype(bf).astype(np.float32).astype(np.float64)

    x = np.asarray(x, np.float32)
    xq = x.astype(bf)  # matches device input exactly
    b2t = np.zeros((128, NOJ), dtype=np.float32)
    for j in range(NOJ):
        mo = min(128, OUT - j * 128)
        b2t[:mo, j] = fc2_b[j * 128 : j * 128 + mo]
    shared = {
        "fc2wT": np.ascontiguousarray(fc2_w.astype(np.float32).T).astype(bf),
        "fc2bt": b2t,
    }
    return Mq, xq, shared


def _in_maps(x, interp_W, head_mask, conv_w, bn1_g, bn1_b, fc1_w,
             bn2_g, bn2_b, fc2_w, fc2_b):
    bf = ml_dtypes.bfloat16
    Mq, xq, shared = _host_prep(
        np.asarray(x), np.asarray(interp_W), np.asarray(head_mask),
        np.asarray(conv_w), np.asarray(bn1_g), np.asarray(bn1_b),
        np.asarray(fc1_w), np.asarray(fc2_w), np.asarray(fc2_b),
    )
    shared["bn2g"] = np.ascontiguousarray(
        np.asarray(bn2_g, np.float32).reshape(4, 128).T
    )
    shared["bn2b"] = np.ascontiguousarray(
        np.asarray(bn2_b, np.float32).reshape(4, 128).T
    )
    fc1_wf = np.asarray(fc1_w, np.float32).astype(np.float64)
    bn1_gf = np.asarray(bn1_g, np.float64)
    bn1_bf = np.asarray(bn1_b, np.float64)
    ch_of_col = np.arange(YF) // NPIX

    in_maps = []
    for c in CORE_IDS:
        xc = xq[c * BL : (c + 1) * BL].astype(np.float32).astype(np.float64)
        # analytic per-shard BN1 stats from x moments
        R = (xc.T @ xc) / BL                     # [64, 64]
        mu_x = xc.mean(axis=0)                   # [64]
        mean_rows = mu_x @ Mq                    # [1152]
        ey2_rows = np.einsum("ar,ab,br->r", Mq, R, Mq, optimize=True)
        mean_c = mean_rows.reshape(NCH, NPIX).mean(1)
        ey2_c = ey2_rows.reshape(NCH, NPIX).mean(1)
        var_c = ey2_c - mean_c**2
        s_c = bn1_gf / np.sqrt(var_c + EPS)      # BN1 scale per channel
        t_c = bn1_bf - mean_c * s_c              # BN1 shift per channel
        sgn = np.where(s_c < 0, -1.0, 1.0)
        a_c = np.abs(s_c)
        a_safe = np.maximum(a_c, 1e-20)
        b_c = t_c / a_safe                       # pre-relu shift
        # Mh: [65, 1152] — rows 0-63 = sgn_c * M, row 64 = b_c per column
        Mh = np.empty((C_IN + 1, YF), dtype=np.float64)
        Mh[:C_IN, :] = Mq * sgn[ch_of_col][None, :]
        Mh[C_IN, :] = b_c[ch_of_col]
        # fc1 weights folded with a_c per input column, transposed
        w1f = fc1_wf * a_c[ch_of_col][None, :]
        m = dict(shared)
        m["mh"] = Mh.astype(np.float32).astype(bf)
        m["fc1wT"] = np.ascontiguousarray(w1f.astype(np.float32).T).astype(bf)
        xh = np.empty((C_IN + 1, BL), dtype=np.float32)
        xh[:C_IN, :] = xq[c * BL : (c + 1) * BL].astype(np.float32).T
        xh[C_IN, :] = 1.0
        m["xhT"] = xh.astype(bf)
        in_maps.append(m)
    return in_maps


def kernel(x, interp_W, head_mask, conv_w, conv_b, bn1_g, bn1_b,
           fc1_w, fc1_b, bn2_g, bn2_b, fc2_w, fc2_b):
    in_maps = _in_maps(x, interp_W, head_mask, conv_w, bn1_g, bn1_b, fc1_w,
                       bn2_g, bn2_b, fc2_w, fc2_b)
    if "nc" not in _CACHE:
        _CACHE["nc"] = _build()
    nc = _CACHE["nc"]
    res = run_bass_kernel_spmd(nc, in_maps, CORE_IDS, trace=TRACE, tmpdir=TRACE_DIR)
    _CACHE["last_res"] = res
    out = np.empty((B, OUT), dtype=np.float32)
    for c in CORE_IDS:
        out[c * BL : (c + 1) * BL, :] = res.results[c]["out"].T.astype(np.float32)
    return out
